# revision 1
# baseline (speedup 1.0000x reference)
"""DeepSeek-MoE layer on 8 TRN2 NeuronCores.

Strategy (expert-parallel, host-side dispatch):
  - Router (x @ gate_w.T, softmax, top-2) computed on host — it *is* the
    sharding decision (~0.02% of total FLOPs).
  - Core c computes routed expert c's SwiGLU FFN over the tokens routed to
    it (gathered+padded to a fixed capacity), plus a 512-token chunk of
    shared expert c//4 (each shared expert covers all 2048 tokens, split
    over 4 cores).
  - All matmuls in bf16 (fp32 PSUM accumulation); weights pre-transposed on
    host into lhsT layout. Combine weights / scatter-add applied on host in
    fp32.
"""
import os
import sys
import types

import numpy as np
import ml_dtypes

import concourse.bass as bass
import concourse.tile as tile
import concourse.mybir as mybir
from concourse import bacc
from concourse.bass_utils import run_bass_kernel_spmd

# ---- problem constants (DeepSeekMoE: B=2,S=1024,H=768,I=3072,E=8,NS=2,k=2) --
H = 768          # hidden
I = 3072         # intermediate
E = 8            # routed experts
NS = 2           # shared experts
TOP_K = 2
N_CORES = 8
KH = H // 128    # 6 k-tiles over H
KI = I // 128    # 24 k-tiles over I
CS = 2048 * NS // N_CORES  # shared-expert tokens per core = 512

BF16 = mybir.dt.bfloat16
F32 = mybir.dt.float32
_bf = ml_dtypes.bfloat16


def _install_ntff_hook():
    """Provide antenv.axon_hooks (missing on this image) so trace=True works."""
    if "antenv.axon_hooks" in sys.modules:
        return
    try:
        from trn_agent_boot.trn_boot import _ntff_profile_via_ctypes
        hook = _ntff_profile_via_ctypes("/opt/axon/libaxon_pjrt.so")
    except Exception:
        hook = None
    mod = types.ModuleType("antenv.axon_hooks")
    mod.get_axon_ntff_profile_hook = lambda: hook
    sys.modules["antenv.axon_hooks"] = mod


def _ffn_job(nc, wpool, hpool, sgpool, gupool, ypool, ystage,
             x_sb, wg_dram, wu_dram, wd_dram, y_dram, n_tiles):
    """Emit one SwiGLU FFN: y = (silu(x Wg) * (x Wu)) Wd for one expert.

    x_sb:    SBUF tile [128, KH, C] bf16 (tokens on free axis)
    wg/wu_dram: DRAM [H, I] bf16 (pre-transposed lhsT layout)
    wd_dram: DRAM [I, H] bf16
    y_dram:  DRAM [H, C] f32 output
    n_tiles: list of (n0, nsz) token column tiles
    """
    # --- weight loads (one slot tag; slots are [128, 3072] bf16) ---
    wg = []
    wu = []
    for k in range(KH):
        t = wpool.tile([128, I], BF16, tag="w")
        nc.sync.dma_start(out=t, in_=wg_dram[k * 128:(k + 1) * 128, :])
        wg.append(t)
    for k in range(KH):
        t = wpool.tile([128, I], BF16, tag="w")
        nc.sync.dma_start(out=t, in_=wu_dram[k * 128:(k + 1) * 128, :])
        wu.append(t)
    wd = []
    for c in range(KI // 4):
        t = wpool.tile([128, 4, H], BF16, tag="w")
        nc.sync.dma_start(
            out=t,
            in_=wd_dram[c * 512:(c + 1) * 512, :].rearrange(
                "(k p) i -> p k i", p=128),
        )
        wd.append(t)

    for (n0, nsz) in n_tiles:
        # --- gate/up + silu*mul, producing h^T [I, nsz] bf16 as 24 tiles ---
        h_tiles = []
        for mi in range(KI):
            g = gupool.tile([128, 512], F32, tag="gu")
            u = gupool.tile([128, 512], F32, tag="gu")
            for k in range(KH):
                nc.tensor.matmul(
                    g[:, :nsz], wg[k][:, mi * 128:(mi + 1) * 128],
                    x_sb[:, k, n0:n0 + nsz],
                    start=(k == 0), stop=(k == KH - 1))
            for k in range(KH):
                nc.tensor.matmul(
                    u[:, :nsz], wu[k][:, mi * 128:(mi + 1) * 128],
                    x_sb[:, k, n0:n0 + nsz],
                    start=(k == 0), stop=(k == KH - 1))
            sg = sgpool.tile([128, 512], F32, tag="sg")
            nc.scalar.activation(sg[:, :nsz], g[:, :nsz],
                                 mybir.ActivationFunctionType.Silu)
            h = hpool.tile([128, 512], BF16, tag="h")
            nc.vector.tensor_mul(h[:, :nsz], sg[:, :nsz], u[:, :nsz])
            h_tiles.append(h)

        # --- down proj: y^T [H, nsz] f32, hj outer to keep PSUM use low ---
        for hj in range(KH):
            yp = ypool.tile([128, 512], F32, tag="y")
            for mi in range(KI):
                nc.tensor.matmul(
                    yp[:, :nsz],
                    wd[mi // 4][:, mi % 4, hj * 128:(hj + 1) * 128],
                    h_tiles[mi][:, :nsz],
                    start=(mi == 0), stop=(mi == KI - 1))
            yst = ystage.tile([128, 512], F32, tag="yst")
            nc.scalar.copy(yst[:, :nsz], yp[:, :nsz])
            nc.sync.dma_start(
                out=y_dram[hj * 128:(hj + 1) * 128, n0:n0 + nsz],
                in_=yst[:, :nsz])


def build_nc(cr):
    """Build the SPMD program. cr = routed-token capacity (multiple of 64)."""
    nc = bacc.Bacc(None, target_bir_lowering=False)
    xr = nc.dram_tensor("xr", [H, cr], BF16, kind="ExternalInput")
    xs = nc.dram_tensor("xs", [H, CS], BF16, kind="ExternalInput")
    wgu = nc.dram_tensor("wgu", [4, H, I], BF16, kind="ExternalInput")
    wd = nc.dram_tensor("wd", [2, I, H], BF16, kind="ExternalInput")
    yr = nc.dram_tensor("yr", [H, cr], F32, kind="ExternalOutput")
    ys = nc.dram_tensor("ys", [H, CS], F32, kind="ExternalOutput")

    r_tiles = [(i * 512, min(512, cr - i * 512)) for i in range((cr + 511) // 512)]
    s_tiles = [(i * 512, min(512, CS - i * 512)) for i in range((CS + 511) // 512)]

    with tile.TileContext(nc) as tc:
        with tc.tile_pool(name="wpool", bufs=20) as wpool, \
             tc.tile_pool(name="xpool", bufs=1) as xpool, \
             tc.tile_pool(name="hpool", bufs=28) as hpool, \
             tc.tile_pool(name="sgpool", bufs=4) as sgpool, \
             tc.tile_pool(name="ystage", bufs=4) as ystage, \
             tc.tile_pool(name="gupool", bufs=4, space="PSUM") as gupool, \
             tc.tile_pool(name="ypool", bufs=2, space="PSUM") as ypool:
            xr_sb = xpool.tile([128, KH, cr], BF16, tag="xr")
            nc.sync.dma_start(
                out=xr_sb, in_=xr.rearrange("(k p) n -> p k n", p=128))
            xs_sb = xpool.tile([128, KH, CS], BF16, tag="xs")
            nc.sync.dma_start(
                out=xs_sb, in_=xs.rearrange("(k p) n -> p k n", p=128))

            _ffn_job(nc, wpool, hpool, sgpool, gupool, ypool, ystage,
                     xr_sb, wgu[0, :, :], wgu[1, :, :], wd[0, :, :],
                     yr, r_tiles)
            _ffn_job(nc, wpool, hpool, sgpool, gupool, ypool, ystage,
                     xs_sb, wgu[2, :, :], wgu[3, :, :], wd[1, :, :],
                     ys, s_tiles)
    nc.finalize()
    return nc


_NC_CACHE = {}


def kernel(hidden_states, gate_w, shared_gate, shared_up, shared_down,
           routed_gate, routed_up, routed_down):
    B, S, _ = hidden_states.shape
    T = B * S
    x = np.asarray(hidden_states, np.float32).reshape(T, H)

    # ---- host router (mirrors reference math; fp64 softmax for stability) --
    logits = x @ np.asarray(gate_w, np.float32).T                    # [T, E]
    lg = logits.astype(np.float64)
    sc = np.exp(lg - lg.max(1, keepdims=True))
    sc /= sc.sum(1, keepdims=True)
    topk_idx = np.argsort(-sc, axis=1, kind="stable")[:, :TOP_K]     # [T, k]
    topk_w = np.take_along_axis(sc, topk_idx, axis=1)
    topk_w = topk_w / (topk_w.sum(1, keepdims=True) + 1e-8)          # [T, k]

    tok_lists = []
    tok_weights = []
    for e in range(E):
        sel = (topk_idx == e)
        toks = np.where(sel.any(1))[0]
        w = (topk_w * sel)[toks].sum(1).astype(np.float32)
        tok_lists.append(toks)
        tok_weights.append(w)
    max_n = max(len(t) for t in tok_lists)
    cr = max(64, -(-max_n // 64) * 64)

    # ---- per-core inputs -------------------------------------------------
    x_bf = x.astype(_bf)
    sg_t = [np.ascontiguousarray(np.asarray(shared_gate[s], np.float32).T).astype(_bf) for s in range(NS)]
    su_t = [np.ascontiguousarray(np.asarray(shared_up[s], np.float32).T).astype(_bf) for s in range(NS)]
    sd_t = [np.ascontiguousarray(np.asarray(shared_down[s], np.float32).T).astype(_bf) for s in range(NS)]

    in_maps = []
    for c in range(N_CORES):
        toks = tok_lists[c]
        xr = np.zeros((H, cr), _bf)
        xr[:, :len(toks)] = x_bf[toks].T
        s = c // (N_CORES // NS)
        q = c % (N_CORES // NS)
        stoks = slice(q * CS, (q + 1) * CS)
        xs_ = np.ascontiguousarray(x_bf[stoks].T)
        rg = np.ascontiguousarray(np.asarray(routed_gate[c], np.float32).T).astype(_bf)
        ru = np.ascontiguousarray(np.asarray(routed_up[c], np.float32).T).astype(_bf)
        rd = np.ascontiguousarray(np.asarray(routed_down[c], np.float32).T).astype(_bf)
        wgu = np.stack([rg, ru, sg_t[s], su_t[s]])                   # [4,H,I]
        wdd = np.stack([rd, sd_t[s]])                                # [2,I,H]
        in_maps.append({"xr": xr, "xs": xs_, "wgu": wgu, "wd": wdd})

    # ---- build + run on 8 cores -----------------------------------------
    if cr not in _NC_CACHE:
        _NC_CACHE[cr] = build_nc(cr)
    nc = _NC_CACHE[cr]

    trace = bool(int(os.environ.get("MOE_TRACE", "0")))
    kw = {}
    if trace:
        _install_ntff_hook()
        kw = dict(trace=True, trace_cores=list(range(N_CORES)))
    res = run_bass_kernel_spmd(nc, in_maps, core_ids=list(range(N_CORES)), **kw)
    if trace:
        print(f"HW exec time: {res.exec_time_ns} ns")
        if res.per_core_scope_times:
            kernel._last_exec_ns = res.exec_time_ns

    # ---- host combine ----------------------------------------------------
    out = np.zeros((T, H), np.float32)
    for c in range(N_CORES):
        toks = tok_lists[c]
        yrT = res.results[c]["yr"]                                   # [H, cr]
        out[toks] += yrT[:, :len(toks)].T * tok_weights[c][:, None]
        q = c % (N_CORES // NS)
        out[q * CS:(q + 1) * CS] += res.results[c]["ys"].T / NS
    return out.reshape(B, S, H)


# revision 2
# speedup vs baseline: 1.0451x; 1.0451x over previous
"""DeepSeek-MoE layer on 8 TRN2 NeuronCores.

Strategy (expert-parallel, host-side dispatch):
  - Router (x @ gate_w.T, softmax, top-2) computed on host — it *is* the
    sharding decision (~0.02% of total FLOPs).
  - Core c computes routed expert c's SwiGLU FFN over the tokens routed to
    it (gathered+padded to a fixed capacity), plus a 512-token chunk of
    shared expert c//4 (each shared expert covers all 2048 tokens, split
    over 4 cores).
  - All matmuls in bf16 (fp32 PSUM accumulation); weights pre-transposed on
    host into lhsT layout. Combine weights / scatter-add applied on host in
    fp32.
Layout per core: tokens live on the matmul free axis (x is stored
transposed [H, C]); weights stream HBM->SBUF in [128, 3072] slots; h =
silu(x Wg) * (x Wu) is staged bf16 in SBUF; down-proj accumulates 24
k-tiles in PSUM; outputs stored transposed [H, C] fp32.
"""
import os
import sys
import types

import numpy as np
import ml_dtypes

import concourse.bass as bass
import concourse.tile as tile
import concourse.mybir as mybir
from concourse import bacc
from concourse.bass_utils import run_bass_kernel_spmd

# ---- problem constants (DeepSeekMoE: B=2,S=1024,H=768,I=3072,E=8,NS=2,k=2) --
H = 768          # hidden
I = 3072         # intermediate
E = 8            # routed experts
NS = 2           # shared experts
TOP_K = 2
N_CORES = 8
KH = H // 128    # 6 k-tiles over H
KI = I // 128    # 24 k-tiles over I
CS = 2048 * NS // N_CORES  # shared-expert tokens per core = 512

BF16 = mybir.dt.bfloat16
F32 = mybir.dt.float32
_bf = ml_dtypes.bfloat16


def _install_ntff_hook():
    """Provide antenv.axon_hooks (missing on this image) so trace=True works."""
    if "antenv.axon_hooks" in sys.modules:
        return
    try:
        from trn_agent_boot.trn_boot import _ntff_profile_via_ctypes
        hook = _ntff_profile_via_ctypes("/opt/axon/libaxon_pjrt.so")
    except Exception:
        hook = None
    mod = types.ModuleType("antenv.axon_hooks")
    mod.get_axon_ntff_profile_hook = lambda: hook
    sys.modules["antenv.axon_hooks"] = mod


def _col_tiles(c):
    """Split c columns into matmul N-tiles. Equal twin tiles beat a
    512+remainder split: per-MM cost is max(N/2.4GHz, ~64ns floor), so a
    small tail tile pays the floor for a full 432-MM lattice."""
    if c <= 512:
        return [(0, c)]
    half = (c // 2 + 31) // 32 * 32
    return [(0, half), (half, c - half)]


def _ffn_job(nc, wpool, hpool, sgpool, gupool, ypool, ystage,
             x_sb, wg_dram, wu_dram, wd_dram, y_dram, n_tiles):
    """Emit one SwiGLU FFN: y = (silu(x Wg) * (x Wu)) Wd for one expert.

    x_sb:    SBUF tile [128, KH, C] bf16 (tokens on free axis)
    wg/wu_dram: DRAM [H, I] bf16 (pre-transposed lhsT layout)
    wd_dram: DRAM [I, H] bf16
    y_dram:  DRAM [H, C] f32 output
    n_tiles: list of (n0, nsz) token column tiles
    """
    # --- weight loads, in consumption order (sync HWDGE ring is FIFO) ---
    wg = []
    wu = []
    for k in range(KH):
        t = wpool.tile([128, I], BF16, tag="w")
        nc.sync.dma_start(out=t, in_=wg_dram[k * 128:(k + 1) * 128, :])
        wg.append(t)
    for k in range(KH):
        t = wpool.tile([128, I], BF16, tag="w")
        nc.sync.dma_start(out=t, in_=wu_dram[k * 128:(k + 1) * 128, :])
        wu.append(t)
    wd = []
    for c in range(KI // 4):
        t = wpool.tile([128, 4, H], BF16, tag="w")
        nc.sync.dma_start(
            out=t,
            in_=wd_dram[c * 512:(c + 1) * 512, :].rearrange(
                "(k p) i -> p k i", p=128),
        )
        wd.append(t)

    # --- phase A: gate/up + silu*mul for ALL column tiles ----------------
    # (finishing all gate/up before any down releases wg/wu slots as early
    # as possible for the next expert's prefetch)
    h_tiles = {}
    for ti, (n0, nsz) in enumerate(n_tiles):
        for mi in range(KI):
            g = gupool.tile([128, 512], F32, tag="gu")
            u = gupool.tile([128, 512], F32, tag="gu")
            for k in range(KH):
                nc.tensor.matmul(
                    g[:, :nsz], wg[k][:, mi * 128:(mi + 1) * 128],
                    x_sb[:, k, n0:n0 + nsz],
                    start=(k == 0), stop=(k == KH - 1))
            for k in range(KH):
                nc.tensor.matmul(
                    u[:, :nsz], wu[k][:, mi * 128:(mi + 1) * 128],
                    x_sb[:, k, n0:n0 + nsz],
                    start=(k == 0), stop=(k == KH - 1))
            sg = sgpool.tile([128, 512], F32, tag="sg")
            nc.scalar.activation(sg[:, :nsz], g[:, :nsz],
                                 mybir.ActivationFunctionType.Silu)
            h = hpool.tile([128, 512], BF16, tag="h")
            nc.vector.tensor_mul(h[:, :nsz], sg[:, :nsz], u[:, :nsz])
            h_tiles[(ti, mi)] = h

    # --- phase B: down proj for all column tiles -------------------------
    for ti, (n0, nsz) in enumerate(n_tiles):
        for hj in range(KH):
            yp = ypool.tile([128, 512], F32, tag="y")
            for mi in range(KI):
                nc.tensor.matmul(
                    yp[:, :nsz],
                    wd[mi // 4][:, mi % 4, hj * 128:(hj + 1) * 128],
                    h_tiles[(ti, mi)][:, :nsz],
                    start=(mi == 0), stop=(mi == KI - 1))
            yst = ystage.tile([128, 512], F32, tag="yst")
            nc.scalar.copy(yst[:, :nsz], yp[:, :nsz])
            # store on the ACT HWDGE ring so stores never head-of-line
            # block the weight loads on the sync ring
            nc.scalar.dma_start(
                out=y_dram[hj * 128:(hj + 1) * 128, n0:n0 + nsz],
                in_=yst[:, :nsz])


def build_nc(cr):
    """Build the SPMD program. cr = routed-token capacity (multiple of 32)."""
    nc = bacc.Bacc(None, target_bir_lowering=False)
    xr = nc.dram_tensor("xr", [H, cr], BF16, kind="ExternalInput")
    xs = nc.dram_tensor("xs", [H, CS], BF16, kind="ExternalInput")
    wgu = nc.dram_tensor("wgu", [4, H, I], BF16, kind="ExternalInput")
    wd = nc.dram_tensor("wd", [2, I, H], BF16, kind="ExternalInput")
    yr = nc.dram_tensor("yr", [H, cr], F32, kind="ExternalOutput")
    ys = nc.dram_tensor("ys", [H, CS], F32, kind="ExternalOutput")

    with tile.TileContext(nc) as tc:
        with tc.tile_pool(name="wpool", bufs=17) as wpool, \
             tc.tile_pool(name="xpool", bufs=1) as xpool, \
             tc.tile_pool(name="hpool", bufs=52) as hpool, \
             tc.tile_pool(name="sgpool", bufs=4) as sgpool, \
             tc.tile_pool(name="ystage", bufs=4) as ystage, \
             tc.tile_pool(name="gupool", bufs=6, space="PSUM") as gupool, \
             tc.tile_pool(name="ypool", bufs=2, space="PSUM") as ypool:
            # x loads split per k-tile so the first matmul only waits for
            # one 128-row slice, not the whole tensor
            xr_sb = xpool.tile([128, KH, cr], BF16, tag="xr")
            xs_sb = xpool.tile([128, KH, CS], BF16, tag="xs")
            for k in range(KH):
                nc.sync.dma_start(
                    out=xr_sb[:, k, :], in_=xr[k * 128:(k + 1) * 128, :])
            for k in range(KH):
                nc.sync.dma_start(
                    out=xs_sb[:, k, :], in_=xs[k * 128:(k + 1) * 128, :])

            _ffn_job(nc, wpool, hpool, sgpool, gupool, ypool, ystage,
                     xr_sb, wgu[0, :, :], wgu[1, :, :], wd[0, :, :],
                     yr, _col_tiles(cr))
            _ffn_job(nc, wpool, hpool, sgpool, gupool, ypool, ystage,
                     xs_sb, wgu[2, :, :], wgu[3, :, :], wd[1, :, :],
                     ys, _col_tiles(CS))
    nc.finalize()
    return nc


_NC_CACHE = {}


def kernel(hidden_states, gate_w, shared_gate, shared_up, shared_down,
           routed_gate, routed_up, routed_down):
    B, S, _ = hidden_states.shape
    T = B * S
    x = np.asarray(hidden_states, np.float32).reshape(T, H)

    # ---- host router (mirrors reference math; fp64 softmax for stability) --
    logits = x @ np.asarray(gate_w, np.float32).T                    # [T, E]
    lg = logits.astype(np.float64)
    sc = np.exp(lg - lg.max(1, keepdims=True))
    sc /= sc.sum(1, keepdims=True)
    topk_idx = np.argsort(-sc, axis=1, kind="stable")[:, :TOP_K]     # [T, k]
    topk_w = np.take_along_axis(sc, topk_idx, axis=1)
    topk_w = topk_w / (topk_w.sum(1, keepdims=True) + 1e-8)          # [T, k]

    tok_lists = []
    tok_weights = []
    for e in range(E):
        sel = (topk_idx == e)
        toks = np.where(sel.any(1))[0]
        w = (topk_w * sel)[toks].sum(1).astype(np.float32)
        tok_lists.append(toks)
        tok_weights.append(w)
    max_n = max(len(t) for t in tok_lists)
    cr = max(64, -(-max_n // 32) * 32)

    # ---- per-core inputs -------------------------------------------------
    x_bf = x.astype(_bf)
    sg_t = [np.ascontiguousarray(np.asarray(shared_gate[s], np.float32).T).astype(_bf) for s in range(NS)]
    su_t = [np.ascontiguousarray(np.asarray(shared_up[s], np.float32).T).astype(_bf) for s in range(NS)]
    sd_t = [np.ascontiguousarray(np.asarray(shared_down[s], np.float32).T).astype(_bf) for s in range(NS)]

    in_maps = []
    for c in range(N_CORES):
        toks = tok_lists[c]
        xr = np.zeros((H, cr), _bf)
        xr[:, :len(toks)] = x_bf[toks].T
        s = c // (N_CORES // NS)
        q = c % (N_CORES // NS)
        stoks = slice(q * CS, (q + 1) * CS)
        xs_ = np.ascontiguousarray(x_bf[stoks].T)
        rg = np.ascontiguousarray(np.asarray(routed_gate[c], np.float32).T).astype(_bf)
        ru = np.ascontiguousarray(np.asarray(routed_up[c], np.float32).T).astype(_bf)
        rd = np.ascontiguousarray(np.asarray(routed_down[c], np.float32).T).astype(_bf)
        wgu = np.stack([rg, ru, sg_t[s], su_t[s]])                   # [4,H,I]
        wdd = np.stack([rd, sd_t[s]])                                # [2,I,H]
        in_maps.append({"xr": xr, "xs": xs_, "wgu": wgu, "wd": wdd})

    # ---- build + run on 8 cores -----------------------------------------
    if cr not in _NC_CACHE:
        _NC_CACHE[cr] = build_nc(cr)
    nc = _NC_CACHE[cr]

    trace = bool(int(os.environ.get("MOE_TRACE", "0")))
    kw = {}
    if trace:
        _install_ntff_hook()
        kw = dict(trace=True, trace_cores=list(range(N_CORES)))
    res = run_bass_kernel_spmd(nc, in_maps, core_ids=list(range(N_CORES)), **kw)
    if trace:
        print(f"HW exec time: {res.exec_time_ns} ns")

    # ---- host combine ----------------------------------------------------
    out = np.zeros((T, H), np.float32)
    for c in range(N_CORES):
        toks = tok_lists[c]
        yrT = res.results[c]["yr"]                                   # [H, cr]
        out[toks] += yrT[:, :len(toks)].T * tok_weights[c][:, None]
        q = c % (N_CORES // NS)
        out[q * CS:(q + 1) * CS] += res.results[c]["ys"].T / NS
    return out.reshape(B, S, H)


# revision 7
# speedup vs baseline: 1.0952x; 1.0480x over previous
"""DeepSeek-MoE layer on 8 TRN2 NeuronCores.

Strategy (expert-parallel, host-side dispatch):
  - Router (x @ gate_w.T, softmax, top-2) computed on host — it *is* the
    sharding decision (~0.02% of total FLOPs).
  - Core c computes routed expert c's SwiGLU FFN over the tokens routed to
    it (gathered+padded to a fixed capacity), plus a 512-token chunk of
    shared expert c//4 (each shared expert covers all 2048 tokens, split
    over 4 cores).
  - All matmuls in bf16 (fp32 PSUM accumulation); weights pre-transposed on
    host into lhsT layout. Combine weights / scatter-add applied on host in
    fp32.
Layout per core: tokens live on the matmul free axis (x is stored
transposed [H, C]); weights stream HBM->SBUF in [128, 3072] slots; h =
silu(x Wg) * (x Wu) is staged bf16 in SBUF; down-proj accumulates 24
k-tiles in PSUM; outputs stored transposed [H, C] fp32.
"""
import os
import sys
import types

import numpy as np
import ml_dtypes

import concourse.bass as bass
import concourse.tile as tile
import concourse.mybir as mybir
from concourse import bacc
from concourse.bass_utils import run_bass_kernel_spmd

# ---- problem constants (DeepSeekMoE: B=2,S=1024,H=768,I=3072,E=8,NS=2,k=2) --
H = 768          # hidden
I = 3072         # intermediate
E = 8            # routed experts
NS = 2           # shared experts
TOP_K = 2
N_CORES = 8
KH = H // 128    # 6 k-tiles over H
KI = I // 128    # 24 k-tiles over I
CS = 2048 * NS // N_CORES  # shared-expert tokens per core = 512

BF16 = mybir.dt.bfloat16
F32 = mybir.dt.float32
_bf = ml_dtypes.bfloat16


def _install_ntff_hook():
    """Provide antenv.axon_hooks (missing on this image) so trace=True works."""
    if "antenv.axon_hooks" in sys.modules:
        return
    try:
        from trn_agent_boot.trn_boot import _ntff_profile_via_ctypes
        hook = _ntff_profile_via_ctypes("/opt/axon/libaxon_pjrt.so")
    except Exception:
        hook = None
    mod = types.ModuleType("antenv.axon_hooks")
    mod.get_axon_ntff_profile_hook = lambda: hook
    sys.modules["antenv.axon_hooks"] = mod


def _col_tiles(c):
    """Split c columns into matmul N-tiles. Equal twin tiles beat a
    512+remainder split: per-MM cost is max(N/2.4GHz, ~64ns floor), so a
    small tail tile pays the floor for a full 432-MM lattice."""
    if c <= 512:
        return [(0, c)]
    half = (c // 2 + 31) // 32 * 32
    return [(0, half), (half, c - half)]


def _ffn_job(nc, wpool, hpool, sgpool, gupool, ypool, ystage,
             x_sb, wg_dram, wu_dram, wd_dram, y_dram, n_tiles,
             sg0pool=None):
    """Emit one SwiGLU FFN: y = (silu(x Wg) * (x Wu)) Wd for one expert.

    x_sb:    SBUF tile [128, KH, C] bf16 (tokens on free axis)
    wg/wu_dram: DRAM [H, I] bf16 (pre-transposed lhsT layout)
    wd_dram: DRAM [I, H] bf16
    y_dram:  DRAM [H, C] f32 output
    n_tiles: list of (n0, nsz) token column tiles
    sg0pool: if given, run gate-first: all gate matmuls (which need only
        wg) before any up matmuls, staging silu(g) as bf16 — keeps the PE
        busy while wu is still streaming in at kernel start.
    """
    # --- weight loads, in consumption order (sync HWDGE ring is FIFO) ---
    wg = []
    wu = []
    for k in range(KH):
        t = wpool.tile([128, I], BF16, tag="w")
        nc.sync.dma_start(out=t, in_=wg_dram[k * 128:(k + 1) * 128, :])
        wg.append(t)
    for k in range(KH):
        t = wpool.tile([128, I], BF16, tag="w")
        nc.sync.dma_start(out=t, in_=wu_dram[k * 128:(k + 1) * 128, :])
        wu.append(t)
    wd = []
    for c in range(KI // 4):
        t = wpool.tile([128, 4, H], BF16, tag="w")
        nc.sync.dma_start(
            out=t,
            in_=wd_dram[c * 512:(c + 1) * 512, :].rearrange(
                "(k p) i -> p k i", p=128),
        )
        wd.append(t)

    # --- phase A: gate/up + silu*mul for ALL column tiles ----------------
    # (finishing all gate/up before any down releases wg/wu slots as early
    # as possible for the next expert's prefetch)
    h_tiles = {}
    for ti, (n0, nsz) in enumerate(n_tiles):
        if sg0pool is not None:
            # gate-first: 144 gate MMs need only wg; they cover the DMA
            # window in which wu is still arriving.
            sgb = {}
            for mi in range(KI):
                g = gupool.tile([128, 512], F32, tag="gu")
                for k in range(KH):
                    nc.tensor.matmul(
                        g[:, :nsz], wg[k][:, mi * 128:(mi + 1) * 128],
                        x_sb[:, k, n0:n0 + nsz],
                        start=(k == 0), stop=(k == KH - 1))
                sg = sg0pool.tile([128, max(n for _, n in n_tiles)],
                                  BF16, tag="sg0")
                nc.scalar.activation(sg[:, :nsz], g[:, :nsz],
                                     mybir.ActivationFunctionType.Silu)
                sgb[mi] = sg
            for mi in range(KI):
                u = gupool.tile([128, 512], F32, tag="gu")
                for k in range(KH):
                    nc.tensor.matmul(
                        u[:, :nsz], wu[k][:, mi * 128:(mi + 1) * 128],
                        x_sb[:, k, n0:n0 + nsz],
                        start=(k == 0), stop=(k == KH - 1))
                h = hpool.tile([128, 512], BF16, tag="h")
                nc.vector.tensor_mul(h[:, :nsz], sgb[mi][:, :nsz],
                                     u[:, :nsz])
                h_tiles[(ti, mi)] = h
        else:
            for mi in range(KI):
                g = gupool.tile([128, 512], F32, tag="gu")
                u = gupool.tile([128, 512], F32, tag="gu")
                for k in range(KH):
                    nc.tensor.matmul(
                        g[:, :nsz], wg[k][:, mi * 128:(mi + 1) * 128],
                        x_sb[:, k, n0:n0 + nsz],
                        start=(k == 0), stop=(k == KH - 1))
                for k in range(KH):
                    nc.tensor.matmul(
                        u[:, :nsz], wu[k][:, mi * 128:(mi + 1) * 128],
                        x_sb[:, k, n0:n0 + nsz],
                        start=(k == 0), stop=(k == KH - 1))
                sg = sgpool.tile([128, 512], F32, tag="sg")
                nc.scalar.activation(sg[:, :nsz], g[:, :nsz],
                                     mybir.ActivationFunctionType.Silu)
                h = hpool.tile([128, 512], BF16, tag="h")
                nc.vector.tensor_mul(h[:, :nsz], sg[:, :nsz], u[:, :nsz])
                h_tiles[(ti, mi)] = h

    # --- phase B: down proj for all column tiles -------------------------
    for ti, (n0, nsz) in enumerate(n_tiles):
        for hj in range(KH):
            yp = ypool.tile([128, 512], F32, tag="y")
            for mi in range(KI):
                nc.tensor.matmul(
                    yp[:, :nsz],
                    wd[mi // 4][:, mi % 4, hj * 128:(hj + 1) * 128],
                    h_tiles[(ti, mi)][:, :nsz],
                    start=(mi == 0), stop=(mi == KI - 1))
            yst = ystage.tile([128, 512], F32, tag="yst")
            nc.scalar.copy(yst[:, :nsz], yp[:, :nsz])
            # store on the ACT HWDGE ring so stores never head-of-line
            # block the weight loads on the sync ring
            nc.scalar.dma_start(
                out=y_dram[hj * 128:(hj + 1) * 128, n0:n0 + nsz],
                in_=yst[:, :nsz])


def build_nc(cr):
    """Build the SPMD program. cr = routed-token capacity (multiple of 32)."""
    nc = bacc.Bacc(None, target_bir_lowering=False)
    xr = nc.dram_tensor("xr", [H, cr], BF16, kind="ExternalInput")
    xs = nc.dram_tensor("xs", [H, CS], BF16, kind="ExternalInput")
    wgu = nc.dram_tensor("wgu", [4, H, I], BF16, kind="ExternalInput")
    wd = nc.dram_tensor("wd", [2, I, H], BF16, kind="ExternalInput")
    yr = nc.dram_tensor("yr", [H, cr], F32, kind="ExternalOutput")
    ys = nc.dram_tensor("ys", [H, CS], F32, kind="ExternalOutput")

    with tile.TileContext(nc) as tc:
        with tc.tile_pool(name="wpool", bufs=15) as wpool, \
             tc.tile_pool(name="xpool", bufs=1) as xpool, \
             tc.tile_pool(name="hpool", bufs=52) as hpool, \
             tc.tile_pool(name="sgpool", bufs=4) as sgpool, \
             tc.tile_pool(name="sg0pool", bufs=26) as sg0pool, \
             tc.tile_pool(name="ystage", bufs=4) as ystage, \
             tc.tile_pool(name="gupool", bufs=6, space="PSUM") as gupool, \
             tc.tile_pool(name="ypool", bufs=2, space="PSUM") as ypool:
            # x loads split per k-tile so the first matmul only waits for
            # one 128-row slice, not the whole tensor
            xr_sb = xpool.tile([128, KH, cr], BF16, tag="xr")
            xs_sb = xpool.tile([128, KH, CS], BF16, tag="xs")
            for k in range(KH):
                nc.sync.dma_start(
                    out=xr_sb[:, k, :], in_=xr[k * 128:(k + 1) * 128, :])
            for k in range(KH):
                nc.sync.dma_start(
                    out=xs_sb[:, k, :], in_=xs[k * 128:(k + 1) * 128, :])

            _ffn_job(nc, wpool, hpool, sgpool, gupool, ypool, ystage,
                     xr_sb, wgu[0, :, :], wgu[1, :, :], wd[0, :, :],
                     yr, _col_tiles(cr), sg0pool=sg0pool)
            _ffn_job(nc, wpool, hpool, sgpool, gupool, ypool, ystage,
                     xs_sb, wgu[2, :, :], wgu[3, :, :], wd[1, :, :],
                     ys, _col_tiles(CS))
    nc.finalize()
    return nc


_NC_CACHE = {}


def kernel(hidden_states, gate_w, shared_gate, shared_up, shared_down,
           routed_gate, routed_up, routed_down):
    B, S, _ = hidden_states.shape
    T = B * S
    x = np.asarray(hidden_states, np.float32).reshape(T, H)

    # ---- host router (mirrors reference math; fp64 softmax for stability) --
    logits = x @ np.asarray(gate_w, np.float32).T                    # [T, E]
    lg = logits.astype(np.float64)
    sc = np.exp(lg - lg.max(1, keepdims=True))
    sc /= sc.sum(1, keepdims=True)
    topk_idx = np.argsort(-sc, axis=1, kind="stable")[:, :TOP_K]     # [T, k]
    topk_w = np.take_along_axis(sc, topk_idx, axis=1)
    topk_w = topk_w / (topk_w.sum(1, keepdims=True) + 1e-8)          # [T, k]

    tok_lists = []
    tok_weights = []
    for e in range(E):
        sel = (topk_idx == e)
        toks = np.where(sel.any(1))[0]
        w = (topk_w * sel)[toks].sum(1).astype(np.float32)
        tok_lists.append(toks)
        tok_weights.append(w)
    max_n = max(len(t) for t in tok_lists)
    cr = max(64, -(-max_n // 32) * 32)

    # ---- per-core inputs -------------------------------------------------
    x_bf = x.astype(_bf)
    sg_t = [np.ascontiguousarray(np.asarray(shared_gate[s], np.float32).T).astype(_bf) for s in range(NS)]
    su_t = [np.ascontiguousarray(np.asarray(shared_up[s], np.float32).T).astype(_bf) for s in range(NS)]
    sd_t = [np.ascontiguousarray(np.asarray(shared_down[s], np.float32).T).astype(_bf) for s in range(NS)]

    in_maps = []
    for c in range(N_CORES):
        toks = tok_lists[c]
        xr = np.zeros((H, cr), _bf)
        xr[:, :len(toks)] = x_bf[toks].T
        s = c // (N_CORES // NS)
        q = c % (N_CORES // NS)
        stoks = slice(q * CS, (q + 1) * CS)
        xs_ = np.ascontiguousarray(x_bf[stoks].T)
        rg = np.ascontiguousarray(np.asarray(routed_gate[c], np.float32).T).astype(_bf)
        ru = np.ascontiguousarray(np.asarray(routed_up[c], np.float32).T).astype(_bf)
        rd = np.ascontiguousarray(np.asarray(routed_down[c], np.float32).T).astype(_bf)
        wgu = np.stack([rg, ru, sg_t[s], su_t[s]])                   # [4,H,I]
        wdd = np.stack([rd, sd_t[s]])                                # [2,I,H]
        in_maps.append({"xr": xr, "xs": xs_, "wgu": wgu, "wd": wdd})

    # ---- build + run on 8 cores -----------------------------------------
    if cr not in _NC_CACHE:
        _NC_CACHE[cr] = build_nc(cr)
    nc = _NC_CACHE[cr]

    trace = bool(int(os.environ.get("MOE_TRACE", "0")))
    kw = {}
    if trace:
        _install_ntff_hook()
        kw = dict(trace=True, trace_cores=list(range(N_CORES)))
    res = run_bass_kernel_spmd(nc, in_maps, core_ids=list(range(N_CORES)), **kw)
    if trace:
        print(f"HW exec time: {res.exec_time_ns} ns")

    # ---- host combine ----------------------------------------------------
    out = np.zeros((T, H), np.float32)
    for c in range(N_CORES):
        toks = tok_lists[c]
        yrT = res.results[c]["yr"]                                   # [H, cr]
        out[toks] += yrT[:, :len(toks)].T * tok_weights[c][:, None]
        q = c % (N_CORES // NS)
        out[q * CS:(q + 1) * CS] += res.results[c]["ys"].T / NS
    return out.reshape(B, S, H)
